# revision 1
# baseline (speedup 1.0000x reference)
"""Trainium2 Bass kernel for 5-relation GAT (nn_GAT_76716705841462).

Strategy: destination-sharded, collective-free.
  * Host prep (sharding only): transpose x, add self-loops, sort each
    relation's edges by destination, bucket into 128-dst windows, pad each
    (window, relation) bucket to (B1+B2)*128 edge slots. Because dma_gather
    indices are int16 (<32768), edge slots are split into B1 "lo" blocks
    gathered from T[0:LOW_CAP] and B2 "hi" blocks gathered from T[H0:], with
    srcs in the overlap band assigned to whichever side has room.
  * Device, phase A (replicated on every core): node table
    T[n] = [h0(128) | 1 | h1(129..257) | 1 | a_src(2) | a_dst(2) | pad] (320 f32)
    via xT_tile.T @ [Wsrc_h0 | 0 | Wsrc_h1 | 0 | Vs | Vd | 0], where
    Vs = per-head W_src @ att_src, Vd = per-head W_dst @ att_dst are built on
    device (broadcast matmul + mult + reduce_sum).  h_dst is never
    materialized; segment-max subtraction is skipped (logits bounded ~8, exp
    is fp32-safe, softmax unchanged).  A per-core side table
    Twin_a[local dst, 64] = [a_dst(2) | pad] is built from xT_local so the
    a_dst gather can use window-local int16 indices.
  * Device, phase B, per (window, relation): dma_gather edge rows from T
    (features + a_src + denominator-ones) and a_dst rows from Twin_a;
    explogit = exp(leaky_relu(a_src+a_dst)) on [128,16] tiles; weighted
    one-hot W_h[e,n] = (iota[n]==dstrel[e]) * explogit[e,h] in one fused
    tensor_scalar(is_equal, mult); TensorE matmul W_h.T @ [G_h | 1]
    accumulates numerator [128,128] and softmax denominator (col 128) in
    PSUM over the 8 blocks.  Divide, accumulate over relations, add 5*bias.
"""

import numpy as np

import concourse.bacc as bacc
import concourse.bass as bass
import concourse.mybir as mybir
import concourse.tile as tile
from concourse.library_config import mlp

P = 128
H = 2
C = 128
D = 256
R = 5
TW = 320          # T row width (f32): 1280B, multiple of 256B for dma_gather
A_OFF = 258       # a_src at 258:260, a_dst at 260:262
AW = 64           # Twin_a row width (f32): 256B
EPS = 1e-16
REL_GROUPS = ((0, 1), (2, 3), (4,))

f32 = mybir.dt.float32
i16 = mybir.dt.int16

_CACHE = {}
_RUN_KWARGS = {}      # test harness may set e.g. {"trace": True}
_LAST_RESULT = None   # BassKernelResults of the last run (for profiling)


def build_program(n_tiles, t_rows, w_pc, B1, B2, low_cap, h0, nw_p,
                  num_devices):
    import os
    ablate = set(os.environ.get("K_ABLATE", "").split(","))
    BT = B1 + B2
    nc = bacc.Bacc("TRN2", target_bir_lowering=False, debug=False,
                   num_devices=num_devices)

    xT = nc.dram_tensor("xT", [D, nw_p], f32, kind="ExternalInput")
    xT_local = nc.dram_tensor("xT_local", [D, w_pc * P], f32,
                              kind="ExternalInput")
    Wsrc = nc.dram_tensor("Wsrc", [D, D], f32, kind="ExternalInput")
    Wdst = nc.dram_tensor("Wdst", [D, D], f32, kind="ExternalInput")
    atts = nc.dram_tensor("atts", [1, D], f32, kind="ExternalInput")
    attd = nc.dram_tensor("attd", [1, D], f32, kind="ExternalInput")
    bias_in = nc.dram_tensor("bias_in", [1, D], f32, kind="ExternalInput")
    iota_in = nc.dram_tensor("iota_in", [P, P], f32, kind="ExternalInput")
    lo_cols = R * B1 * P // 16
    hi_cols = R * B2 * P // 16
    ad_cols = R * BT * P // 16
    lo16 = nc.dram_tensor("lo16", [w_pc * P, lo_cols], i16,
                          kind="ExternalInput")
    hi16 = nc.dram_tensor("hi16", [w_pc * P, hi_cols], i16,
                          kind="ExternalInput")
    ad16 = nc.dram_tensor("ad16", [w_pc * P, ad_cols], i16,
                          kind="ExternalInput")
    drelx = nc.dram_tensor("drelx", [w_pc * P, R * BT], f32,
                           kind="ExternalInput")
    y = nc.dram_tensor("y", [w_pc * P, D], f32, kind="ExternalOutput")

    T = nc.dram_tensor("T", [t_rows, TW], f32)
    Twin_a = nc.dram_tensor("Twin_a", [w_pc * P, AW], f32)

    def grp_cols(per_rel_slots):
        offs, acc = [], 0
        for g in REL_GROUPS:
            offs.append((acc, acc + len(g) * per_rel_slots // 16))
            acc = offs[-1][1]
        return offs

    lo_goff = grp_cols(B1 * P)
    hi_goff = grp_cols(B2 * P)

    # ---- TileContext 1: setup + table build (exit = all-engine barrier) ----
    with tile.TileContext(nc) as tc:
        with (
            tc.tile_pool(name="setup", bufs=1) as su,
            tc.tile_pool(name="ps_su", bufs=1, space="PSUM") as psu,
        ):
            ws_h = [su.tile([P, D], f32, name=f"ws_h{k}") for k in range(2)]
            wd_h = [su.tile([P, D], f32, name=f"wd_h{k}") for k in range(2)]
            for k in range(2):
                nc.sync.dma_start(ws_h[k][:], Wsrc[k * P:(k + 1) * P, :])
                nc.sync.dma_start(wd_h[k][:], Wdst[k * P:(k + 1) * P, :])
            ones1 = su.tile([1, P], f32)
            nc.vector.memset(ones1[:], 1.0)
            atts_sb = su.tile([1, D], f32)
            attd_sb = su.tile([1, D], f32)
            nc.sync.dma_start(atts_sb[:], atts[:])
            nc.sync.dma_start(attd_sb[:], attd[:])
            atts_bc = su.tile([P, D], f32)
            attd_bc = su.tile([P, D], f32)
            for row_sb, bc in ((atts_sb, atts_bc), (attd_sb, attd_bc)):
                ps_bc = psu.tile([P, D], f32, name="ps_bc", tag="ps_bc")
                nc.tensor.matmul(out=ps_bc[:], lhsT=ones1[:], rhs=row_sb[:],
                                 start=True, stop=True)
                nc.vector.tensor_copy(bc[:], ps_bc[:])

            rhs_k = [su.tile([P, TW], f32, name=f"rhs_k{k}") for k in range(2)]
            for k in range(2):
                rk = rhs_k[k]
                nc.vector.memset(rk[:], 0.0)
                nc.vector.tensor_copy(rk[:, 0:C], ws_h[k][:, 0:C])
                nc.vector.tensor_copy(rk[:, C + 1:2 * C + 1], ws_h[k][:, C:D])
                for h in range(H):
                    for src_w, src_bc, col in (
                        (ws_h[k], atts_bc, A_OFF + h),
                        (wd_h[k], attd_bc, A_OFF + 2 + h),
                    ):
                        scratch = su.tile([P, C], f32, name="vscr",
                                          tag="vscr", bufs=2)
                        nc.vector.tensor_tensor(
                            out=scratch[:],
                            in0=src_w[:, h * C:(h + 1) * C],
                            in1=src_bc[:, h * C:(h + 1) * C],
                            op=mybir.AluOpType.mult)
                        nc.vector.tensor_reduce(
                            out=rk[:, col:col + 1], in_=scratch[:],
                            axis=mybir.AxisListType.X,
                            op=mybir.AluOpType.add)

            with (
                tc.tile_pool(name="sb_tbl", bufs=3) as stp,
                tc.tile_pool(name="ps_tbl", bufs=2, space="PSUM") as ptp,
            ):
                for t in range(n_tiles):
                    xk0 = stp.tile([P, P], f32, name="xk0")
                    xk1 = stp.tile([P, P], f32, name="xk1")
                    nc.sync.dma_start(xk0[:], xT[0:P, t * P:(t + 1) * P])
                    nc.sync.dma_start(xk1[:], xT[P:D, t * P:(t + 1) * P])
                    ps_t = ptp.tile([P, TW], f32, name="ps_t")
                    nc.tensor.matmul(out=ps_t[:], lhsT=xk0[:], rhs=rhs_k[0][:],
                                     start=True, stop=False)
                    nc.tensor.matmul(out=ps_t[:], lhsT=xk1[:], rhs=rhs_k[1][:],
                                     start=False, stop=True)
                    stg = stp.tile([P, TW], f32, name="stg")
                    nc.vector.tensor_copy(stg[:], ps_t[:])
                    nc.vector.memset(stg[:, C:C + 1], 1.0)
                    nc.vector.memset(stg[:, 2 * C + 1:2 * C + 2], 1.0)
                    nc.sync.dma_start(T[t * P:(t + 1) * P, :], stg[:])
                for t in range(w_pc):
                    xk0 = stp.tile([P, P], f32, name="xk0")
                    xk1 = stp.tile([P, P], f32, name="xk1")
                    nc.sync.dma_start(xk0[:], xT_local[0:P, t * P:(t + 1) * P])
                    nc.sync.dma_start(xk1[:], xT_local[P:D, t * P:(t + 1) * P])
                    ps_a = ptp.tile([P, 4], f32, name="ps_a")
                    nc.tensor.matmul(out=ps_a[:], lhsT=xk0[:],
                                     rhs=rhs_k[0][:, A_OFF:A_OFF + 4],
                                     start=True, stop=False)
                    nc.tensor.matmul(out=ps_a[:], lhsT=xk1[:],
                                     rhs=rhs_k[1][:, A_OFF:A_OFF + 4],
                                     start=False, stop=True)
                    stga = stp.tile([P, AW], f32, name="stga")
                    nc.vector.memset(stga[:], 0.0)
                    nc.vector.tensor_copy(stga[:, 0:2], ps_a[:, 2:4])
                    nc.sync.dma_start(Twin_a[t * P:(t + 1) * P, :], stga[:])

    # ---- TileContext 2: attention + aggregation ----
    if "nomain" in ablate:
        w_pc = 0
    w_pc = min(w_pc, int(os.environ.get("K_WINCAP", 10**9)))
    with tile.TileContext(nc) as tc:
        with (
            tc.tile_pool(name="su2", bufs=1) as su,
            tc.tile_pool(name="ps_su2", bufs=1, space="PSUM") as psu,
            tc.tile_pool(name="sb_g", bufs=2) as sgp,
            tc.tile_pool(name="sb_w", bufs=4) as swp,
            tc.tile_pool(name="sb_sm", bufs=3) as ssp,
            tc.tile_pool(name="sb_out", bufs=2) as sop,
            tc.tile_pool(name="ps_mm", bufs=2, space="PSUM") as pmp,
        ):
            nc.gpsimd.load_library(mlp)
            iota_t = su.tile([P, P], f32)
            nc.sync.dma_start(iota_t[:], iota_in[:])
            ones1 = su.tile([1, P], f32)
            nc.vector.memset(ones1[:], 1.0)
            bias_sb = su.tile([1, D], f32)
            nc.sync.dma_start(bias_sb[:], bias_in[:])
            bias5 = su.tile([P, D], f32)
            ps_bc = psu.tile([P, D], f32)
            nc.tensor.matmul(out=ps_bc[:], lhsT=ones1[:], rhs=bias_sb[:],
                             start=True, stop=True)
            nc.vector.tensor_scalar_mul(bias5[:], ps_bc[:], float(R))

            for w in range(w_pc):
                rows = slice(w * P, (w + 1) * P)
                drl = ssp.tile([P, R * BT], f32, name="drl")
                nc.sync.dma_start(drl[:], drelx[rows, :])
                lo_t = ssp.tile([P, lo_cols], i16, name="lo_t")
                hi_t = ssp.tile([P, hi_cols], i16, name="hi_t")
                ad_t = ssp.tile([P, ad_cols], i16, name="ad_t")
                nc.sync.dma_start(lo_t[:], lo16[rows, :])
                nc.sync.dma_start(hi_t[:], hi16[rows, :])
                nc.sync.dma_start(ad_t[:], ad16[rows, :])
                outacc = sop.tile([P, D], f32, name="outacc")

                for gi, grp in enumerate(REL_GROUPS):
                    ng = len(grp)
                    G_lo = sgp.tile([P, 2 * B1 * TW], f32, name="G_lo")
                    G_hi = sgp.tile([P, 2 * B2 * TW], f32, name="G_hi")
                    if "nogather" in ablate or "nolo" in ablate:
                        nc.vector.memset(G_lo[:], 0.25)
                    else:
                        nc.gpsimd.dma_gather(
                            out_ap=G_lo[:, :ng * B1 * TW].rearrange(
                                "p (j e) -> p j e", e=TW),
                            in_ap=T[0:low_cap, :],
                            idxs_ap=lo_t[:, lo_goff[gi][0]:lo_goff[gi][1]],
                            num_idxs=ng * B1 * P,
                            num_idxs_reg=ng * B1 * P,
                            elem_size=TW)
                    if "nogather" in ablate or "nohi" in ablate:
                        nc.vector.memset(G_hi[:], 0.25)
                    else:
                        nc.gpsimd.dma_gather(
                            out_ap=G_hi[:, :ng * B2 * TW].rearrange(
                                "p (j e) -> p j e", e=TW),
                            in_ap=T[h0:t_rows, :],
                            idxs_ap=hi_t[:, hi_goff[gi][0]:hi_goff[gi][1]],
                            num_idxs=ng * B2 * P,
                            num_idxs_reg=ng * B2 * P,
                            elem_size=TW)

                    for rl, r in enumerate(grp):
                        if "norel" in ablate:
                            continue
                        G_ad = sgp.tile([P, BT * AW], f32, name="G_ad")
                        if "nogather" in ablate or "noad" in ablate:
                            nc.vector.memset(G_ad[:], 0.25)
                        else:
                            nc.gpsimd.dma_gather(
                                out_ap=G_ad[:].rearrange(
                                    "p (j e) -> p j e", e=AW),
                                in_ap=Twin_a[:],
                                idxs_ap=ad_t[:, r * BT * P // 16:
                                             (r + 1) * BT * P // 16],
                                num_idxs=BT * P,
                                num_idxs_reg=BT * P,
                                elem_size=AW)
                        asum = ssp.tile([P, BT * H], f32, name="asum")
                        nc.vector.tensor_tensor(
                            out=asum[:].rearrange("p (b h) -> p b h", h=H)
                                [:, 0:B1, :],
                            in0=G_lo[:].rearrange("p (j e) -> p j e", e=TW)
                                [:, rl * B1:(rl + 1) * B1, A_OFF:A_OFF + H],
                            in1=G_ad[:].rearrange("p (j e) -> p j e", e=AW)
                                [:, 0:B1, 0:H],
                            op=mybir.AluOpType.add)
                        nc.vector.tensor_tensor(
                            out=asum[:].rearrange("p (b h) -> p b h", h=H)
                                [:, B1:BT, :],
                            in0=G_hi[:].rearrange("p (j e) -> p j e", e=TW)
                                [:, rl * B2:(rl + 1) * B2, A_OFF:A_OFF + H],
                            in1=G_ad[:].rearrange("p (j e) -> p j e", e=AW)
                                [:, B1:BT, 0:H],
                            op=mybir.AluOpType.add)
                        lsc = ssp.tile([P, BT * H], f32, name="lsc")
                        nc.vector.tensor_scalar_mul(lsc[:], asum[:], 0.2)
                        lrl = ssp.tile([P, BT * H], f32, name="lrl")
                        nc.vector.tensor_tensor(
                            out=lrl[:], in0=asum[:], in1=lsc[:],
                            op=mybir.AluOpType.max)
                        expl = ssp.tile([P, BT * H], f32, name="expl")
                        nc.scalar.activation(
                            expl[:], lrl[:], mybir.ActivationFunctionType.Exp)

                        ps_h = [pmp.tile([P, C + 1], f32, name=f"ps_h{h}")
                                for h in range(H)]
                        for b in range(BT):
                            src_tile = G_lo if b < B1 else G_hi
                            bb = b if b < B1 else b - B1
                            base = (rl * (B1 if b < B1 else B2) + bb) * TW
                            for h in range(H):
                                wt = swp.tile([P, P], f32, name="wt", tag="wt")
                                nc.vector.tensor_scalar(
                                    out=wt[:], in0=iota_t[:],
                                    scalar1=drl[:, r * BT + b:r * BT + b + 1],
                                    scalar2=expl[:, b * H + h:b * H + h + 1],
                                    op0=mybir.AluOpType.is_equal,
                                    op1=mybir.AluOpType.mult)
                                nc.tensor.matmul(
                                    out=ps_h[h][:], lhsT=wt[:],
                                    rhs=src_tile[:, base + h * (C + 1):
                                                 base + (h + 1) * (C + 1)],
                                    start=(b == 0), stop=(b == BT - 1))
                        for h in range(H):
                            s_eps = ssp.tile([P, 1], f32, name="s_eps")
                            nc.vector.tensor_scalar_add(
                                s_eps[:], ps_h[h][:, C:C + 1], EPS)
                            recip = ssp.tile([P, 1], f32, name="recip")
                            nc.vector.reciprocal(recip[:], s_eps[:])
                            dst_sl = outacc[:, h * C:(h + 1) * C]
                            if r == 0:
                                nc.vector.tensor_scalar_mul(
                                    dst_sl, ps_h[h][:, 0:C], recip[:, :1])
                            else:
                                tmp = ssp.tile([P, C], f32, name="tmp")
                                nc.vector.tensor_scalar_mul(
                                    tmp[:], ps_h[h][:, 0:C], recip[:, :1])
                                nc.vector.tensor_tensor(
                                    out=dst_sl, in0=dst_sl, in1=tmp[:],
                                    op=mybir.AluOpType.add)
                if "norel" in ablate:
                    nc.vector.memset(outacc[:], 0.0)
                nc.vector.tensor_tensor(out=outacc[:], in0=outacc[:],
                                        in1=bias5[:], op=mybir.AluOpType.add)
                nc.sync.dma_start(y[rows, :], outacc[:])

    nc.finalize()
    return nc

def _wrap16(vals):
    """[n] int array -> 16-partition-wrapped [128, n//16] int16 (replicated)."""
    n = len(vals)
    assert n % 16 == 0
    a = np.asarray(vals, np.int16).reshape(n // 16, 16).T  # [16, n//16]
    return np.tile(a, (8, 1))


def prep_inputs(inputs, ncores, low_cap=32768):
    x = np.asarray(inputs["x"], dtype=np.float32)
    N = x.shape[0]
    nw_real = -(-N // P)
    NW = -(-nw_real // ncores) * ncores
    w_pc = NW // ncores
    n_tiles = nw_real
    t_rows = n_tiles * P
    low_cap = min(low_cap, t_rows)
    h0 = t_rows - low_cap

    rels = ["parent", "child", "precede", "follow", "peer"]
    loops = np.arange(N, dtype=np.int64)
    per_rel = []
    for rn in rels:
        ei = np.asarray(inputs[f"edge_index_{rn}"])
        src = np.concatenate([ei[0], loops]).astype(np.int64)
        dst = np.concatenate([ei[1], loops]).astype(np.int64)
        order = np.argsort(dst, kind="stable")
        src, dst = src[order], dst[order]
        w_of = dst // P
        cnt = np.bincount(w_of, minlength=NW)
        starts = np.zeros(NW + 1, np.int64)
        np.cumsum(cnt, out=starts[1:])
        per_rel.append((src, dst, starts))

    # global B1/B2 from per-(w,r) counts
    must_lo_max = must_hi_max = tot_max = 0
    for src, dst, starts in per_rel:
        for w in range(NW):
            s, e = starts[w], starts[w + 1]
            sw = src[s:e]
            must_lo_max = max(must_lo_max, int((sw < h0).sum()))
            must_hi_max = max(must_hi_max, int((sw >= low_cap).sum()))
            tot_max = max(tot_max, e - s)
    B1 = max(1, -(-must_lo_max // P))
    B2 = max(1, -(-must_hi_max // P), -(-tot_max // P) - B1)
    while B1 * P < must_lo_max or (tot_max - B1 * P) > B2 * P:
        B1 += 1
    BT = B1 + B2

    lo_idx = np.zeros((NW, R, B1 * P), np.int64)
    hi_idx = np.zeros((NW, R, B2 * P), np.int64)  # pad -> hi row 0 (valid)
    ad_idx = np.zeros((NW, R, BT * P), np.int64)
    drelx = np.full((NW, R, BT * P), float(P), np.float32)
    for r, (src, dst, starts) in enumerate(per_rel):
        for w in range(NW):
            s, e = starts[w], starts[w + 1]
            sw, dw = src[s:e], dst[s:e]
            is_lo = sw < h0
            is_hi = sw >= low_cap
            flex = ~is_lo & ~is_hi
            n_lo_strict = int(is_lo.sum())
            room = B1 * P - n_lo_strict
            fi = np.flatnonzero(flex)
            lo_sel = np.concatenate([np.flatnonzero(is_lo), fi[:room]])
            hi_sel = np.concatenate([np.flatnonzero(is_hi), fi[room:]])
            assert len(lo_sel) <= B1 * P and len(hi_sel) <= B2 * P, (
                w, r, len(lo_sel), len(hi_sel))
            lo_idx[w, r, :len(lo_sel)] = sw[lo_sel]
            hi_idx[w, r, :len(hi_sel)] = sw[hi_sel] - h0
            # a_dst indices are window-local+core-local; fill later with dw
            ad_idx[w, r, :len(lo_sel)] = dw[lo_sel]
            ad_idx[w, r, B1 * P:B1 * P + len(hi_sel)] = dw[hi_sel]
            drelx[w, r, :len(lo_sel)] = (dw[lo_sel] - w * P)
            drelx[w, r, B1 * P:B1 * P + len(hi_sel)] = (dw[hi_sel] - w * P)

    nw_p = NW * P
    xT = np.zeros((D, nw_p), np.float32)
    xT[:, :N] = np.ascontiguousarray(x.T)

    shared = {
        "xT": xT,
        "Wsrc": np.ascontiguousarray(np.asarray(inputs["W_src"], np.float32)),
        "Wdst": np.ascontiguousarray(np.asarray(inputs["W_dst"], np.float32)),
        "atts": np.asarray(inputs["att_src"], np.float32).reshape(1, D).copy(),
        "attd": np.asarray(inputs["att_dst"], np.float32).reshape(1, D).copy(),
        "bias_in": np.asarray(inputs["bias"], np.float32).reshape(1, D).copy(),
        "iota_in": np.tile(np.arange(P, dtype=np.float32), (P, 1)),
    }

    def grp_cols(per_rel_slots):
        offs, acc = [], 0
        for g in REL_GROUPS:
            offs.append((acc, acc + len(g) * per_rel_slots // 16))
            acc = offs[-1][1]
        return offs

    percore = []
    for c in range(ncores):
        cb = c * w_pc * P  # first dst node owned by this core
        lo16 = np.zeros((w_pc * P, R * B1 * P // 16), np.int16)
        hi16 = np.zeros((w_pc * P, R * B2 * P // 16), np.int16)
        ad16 = np.zeros((w_pc * P, R * BT * P // 16), np.int16)
        drl = np.zeros((w_pc * P, R * BT), np.float32)
        ad_local = np.clip(ad_idx - cb, 0, w_pc * P - 1)
        for wl in range(w_pc):
            w = c * w_pc + wl
            # idx value order within a group gather: (rel, block, partition)
            def fill(dst_arr, goffs, idx_src, per_rel_slots):
                for gi, grp in enumerate(REL_GROUPS):
                    vals = np.concatenate(
                        [idx_src[w, r, :per_rel_slots] for r in grp])
                    dst_arr[wl * P:(wl + 1) * P,
                            goffs[gi][0]:goffs[gi][1]] = _wrap16(vals)
            fill(lo16, grp_cols(B1 * P), lo_idx, B1 * P)
            fill(hi16, grp_cols(B2 * P), hi_idx, B2 * P)
            fill(ad16, grp_cols(BT * P), ad_local, BT * P)
            # drel columns: [r*BT + b] value for slot block b, this partition
            drl[wl * P:(wl + 1) * P, :] = (
                drelx[w].reshape(R * BT, P).T.reshape(P, R * BT))
        percore.append({
            "lo16": lo16, "hi16": hi16, "ad16": ad16, "drelx": drl,
            "xT_local": np.ascontiguousarray(xT[:, cb:cb + w_pc * P]),
        })
    meta = dict(N=N, NW=NW, w_pc=w_pc, n_tiles=n_tiles, t_rows=t_rows,
                B1=B1, B2=B2, low_cap=low_cap, h0=h0, nw_p=nw_p)
    return meta, shared, percore


def kernel(**inputs):
    global _LAST_RESULT
    from concourse.bass_utils import run_bass_kernel_spmd

    ncores = 8
    meta, shared, percore = prep_inputs(inputs, ncores)
    key = tuple(sorted(meta.items()))
    if key not in _CACHE:
        _CACHE[key] = build_program(
            meta["n_tiles"], meta["t_rows"], meta["w_pc"], meta["B1"],
            meta["B2"], meta["low_cap"], meta["h0"], meta["nw_p"], ncores)
    nc = _CACHE[key]
    in_maps = [dict(shared, **percore[c]) for c in range(ncores)]
    res = run_bass_kernel_spmd(nc, in_maps, core_ids=list(range(ncores)),
                               **_RUN_KWARGS)
    _LAST_RESULT = res
    out = np.concatenate([res.results[c]["y"] for c in range(ncores)], axis=0)
    return np.ascontiguousarray(out[:meta["N"]])



# revision 7
# speedup vs baseline: 1.2588x; 1.2588x over previous
"""Trainium2 Bass kernel for 5-relation GAT (nn_GAT_76716705841462), v2.

Destination-sharded, collective-free, fp16 node table.

Node table T row (fp16, 384 elems = 768B stride, mult of 256B):
  [ad0 ad1 | h0 (2:130) | 1 (130) | h1 (131:259) | 1 (259) | as0 as1 | pad]
One row layout serves three consumers:
  * main gather (768B/edge): features+ones+a_src (+unused ad) by global src,
    split lo/hi so int16 indices fit;
  * a_dst gather (256B/edge): first 256B of the row via elem_step=384 from
    the per-core local slice Tlocal, window-local int16 indices;
  * self-loops: plain DMA of the window's own 128 rows (Tself).
Self-loops are excluded from the edge lists: their contribution (same for
all 5 relations) is computed once per window from Tself and added to every
relation's numerator/denominator.

Per (window, rel): one-hot Ind[e, dst] built for ALL blocks in one DVE
is_equal with broadcast APs; gathered rows scaled in-place by
exp(min(leaky_relu(a_src+a_dst), CLAMP)) (per-head); one 258-wide matmul
per 128-edge block accumulates [num_h0|den_h0|num_h1|den_h1] in PSUM
(denominator rides the embedded "1" columns). Final: den += explS,
reciprocal, num += numS, alpha-scale, reduce over rels, +5*bias.
"""

import numpy as np

import concourse.bacc as bacc
import concourse.bass as bass
import concourse.mybir as mybir
import concourse.tile as tile

P = 128
H = 2
C = 128
D = 256
R = 5
TW = 384          # T row width (fp16): 768B
TU = 262          # used cols
FEAT0 = 2         # h0 at 2:130, one at 130, h1 at 131:259, one at 259
AS0 = 260         # a_src at 260:262
MMW = 2 * (C + 1)  # 258-wide matmul rhs [h0|1|h1|1]
CLAMP = 10.5
NEG = 0.2

f32 = mybir.dt.float32
f16 = mybir.dt.float16
i16 = mybir.dt.int16

_CACHE = {}
_RUN_KWARGS = {}
_LAST_RESULT = None


def build_program(n_tiles, t_rows, w_pc, B1, B2, low_cap, h0, nw_p,
                  num_devices):
    import os
    ablate = set(os.environ.get("K_ABLATE", "").split(","))
    BT = B1 + B2
    NLO = R * B1          # lo blocks per window
    NHI = R * B2
    NAD = R * BT
    lp = w_pc * P         # local rows per core
    nc = bacc.Bacc("TRN2", target_bir_lowering=False, debug=False,
                   num_devices=num_devices)

    xT = nc.dram_tensor("xT", [D, nw_p], f32, kind="ExternalInput")
    xT_local = nc.dram_tensor("xT_local", [D, lp], f32, kind="ExternalInput")
    Wsrc = nc.dram_tensor("Wsrc", [D, D], f32, kind="ExternalInput")
    Wdst = nc.dram_tensor("Wdst", [D, D], f32, kind="ExternalInput")
    atts = nc.dram_tensor("atts", [1, D], f32, kind="ExternalInput")
    attd = nc.dram_tensor("attd", [1, D], f32, kind="ExternalInput")
    bias_in = nc.dram_tensor("bias_in", [1, D], f32, kind="ExternalInput")
    iota_in = nc.dram_tensor("iota_in", [P, P], f16, kind="ExternalInput")
    lo16 = nc.dram_tensor("lo16", [lp, NLO * P // 16], i16,
                          kind="ExternalInput")
    hi16 = nc.dram_tensor("hi16", [lp, NHI * P // 16], i16,
                          kind="ExternalInput")
    ad16 = nc.dram_tensor("ad16", [lp, NAD * P // 16], i16,
                          kind="ExternalInput")
    dlo = nc.dram_tensor("dlo", [lp, NLO], f16, kind="ExternalInput")
    dhi = nc.dram_tensor("dhi", [lp, NHI], f16, kind="ExternalInput")
    y = nc.dram_tensor("y", [lp, D], f32, kind="ExternalOutput")

    T = nc.dram_tensor("T", [t_rows, TW], f16)
    Tlocal = nc.dram_tensor("Tlocal", [lp, TW], f16)

    # ---- TileContext 1: weights setup + node tables ----
    with tile.TileContext(nc) as tc:
        with (
            tc.tile_pool(name="setup", bufs=1) as su,
            tc.tile_pool(name="ps_su", bufs=1, space="PSUM") as psu,
        ):
            ws_h = [su.tile([P, D], f32, name=f"ws_h{k}") for k in range(2)]
            wd_h = [su.tile([P, D], f32, name=f"wd_h{k}") for k in range(2)]
            for k in range(2):
                nc.sync.dma_start(ws_h[k][:], Wsrc[k * P:(k + 1) * P, :])
                nc.sync.dma_start(wd_h[k][:], Wdst[k * P:(k + 1) * P, :])
            ones1 = su.tile([1, P], f32)
            nc.vector.memset(ones1[:], 1.0)
            atts_sb = su.tile([1, D], f32)
            attd_sb = su.tile([1, D], f32)
            nc.sync.dma_start(atts_sb[:], atts[:])
            nc.sync.dma_start(attd_sb[:], attd[:])
            atts_bc = su.tile([P, D], f32)
            attd_bc = su.tile([P, D], f32)
            for row_sb, bc in ((atts_sb, atts_bc), (attd_sb, attd_bc)):
                ps_bc = psu.tile([P, D], f32, name="ps_bc", tag="ps_bc")
                nc.tensor.matmul(out=ps_bc[:], lhsT=ones1[:], rhs=row_sb[:],
                                 start=True, stop=True)
                nc.vector.tensor_copy(bc[:], ps_bc[:])

            # rhs16_k[k]: [P, TU] fp16 matmul rhs producing one T row tile
            rhs16_k = []
            for k in range(2):
                r32 = su.tile([P, TU], f32, name=f"r32_{k}", tag="r32",
                              bufs=2)
                nc.vector.memset(r32[:], 0.0)
                nc.vector.tensor_copy(r32[:, FEAT0:FEAT0 + C],
                                      ws_h[k][:, 0:C])
                nc.vector.tensor_copy(r32[:, FEAT0 + C + 1:FEAT0 + 2 * C + 1],
                                      ws_h[k][:, C:D])
                for h in range(H):
                    for src_w, src_bc, col in (
                        (ws_h[k], atts_bc, AS0 + h),
                        (wd_h[k], attd_bc, h),
                    ):
                        scratch = su.tile([P, C], f32, name="vscr",
                                          tag="vscr", bufs=2)
                        nc.vector.tensor_tensor(
                            out=scratch[:],
                            in0=src_w[:, h * C:(h + 1) * C],
                            in1=src_bc[:, h * C:(h + 1) * C],
                            op=mybir.AluOpType.mult)
                        nc.vector.tensor_reduce(
                            out=r32[:, col:col + 1], in_=scratch[:],
                            axis=mybir.AxisListType.X,
                            op=mybir.AluOpType.add)
                r16 = su.tile([P, TU], f16, name=f"rhs16_{k}")
                nc.vector.tensor_copy(r16[:], r32[:])
                rhs16_k.append(r16)

            # node table build: global T then local Tlocal
            with (
                tc.tile_pool(name="sb_tbl", bufs=3) as stp,
                tc.tile_pool(name="ps_tbl", bufs=4, space="PSUM") as ptp,
            ):
                def build_rows(src_dram, ncols, out_dram):
                    ntile = ncols // P
                    CH = 16       # tiles per strip
                    for c0 in range(0, ntile, CH):
                        cn = min(CH, ntile - c0)
                        w0, w1 = c0 * P, (c0 + cn) * P
                        s32a = stp.tile([P, CH * P], f32, name="s32a")
                        s32b = stp.tile([P, CH * P], f32, name="s32b")
                        nc.sync.dma_start(s32a[:, :cn * P],
                                          src_dram[0:P, w0:w1])
                        nc.sync.dma_start(s32b[:, :cn * P],
                                          src_dram[P:D, w0:w1])
                        s16a = stp.tile([P, CH * P], f16, name="s16a")
                        s16b = stp.tile([P, CH * P], f16, name="s16b")
                        nc.vector.tensor_copy(s16a[:, :cn * P],
                                              s32a[:, :cn * P])
                        nc.vector.tensor_copy(s16b[:, :cn * P],
                                              s32b[:, :cn * P])
                        for g0 in range(0, cn, 4):
                            gn = min(4, cn - g0)
                            stg4 = stp.tile([P, 4 * TW], f16, name="stg4")
                            nc.vector.memset(
                                stg4[:].rearrange("p (j e) -> p j e", e=TW)
                                [:, 0:gn, TU:TW], 0.0)
                            for j in range(gn):
                                t = g0 + j
                                ps_t = ptp.tile([P, TU], f32, name="ps_t")
                                nc.tensor.matmul(
                                    out=ps_t[:],
                                    lhsT=s16a[:, t * P:(t + 1) * P],
                                    rhs=rhs16_k[0][:],
                                    start=True, stop=False)
                                nc.tensor.matmul(
                                    out=ps_t[:],
                                    lhsT=s16b[:, t * P:(t + 1) * P],
                                    rhs=rhs16_k[1][:],
                                    start=False, stop=True)
                                nc.scalar.activation(
                                    stg4[:, j * TW:j * TW + TU], ps_t[:],
                                    mybir.ActivationFunctionType.Copy)
                            ones_ap = stg4[:].rearrange(
                                "p (j e) -> p j e", e=TW)[
                                :, 0:gn, FEAT0 + C:FEAT0 + 2 * C + 2:C + 1]
                            nc.vector.memset(ones_ap, 1.0)
                            r0 = (c0 + g0) * P
                            out_ap = out_dram[
                                r0:r0 + gn * P, :].rearrange(
                                "(j p) e -> p j e", p=P)
                            in_ap = stg4[:, :gn * TW].rearrange(
                                "p (j e) -> p j e", e=TW)
                            nc.sync.dma_start(out_ap, in_ap)

                build_rows(xT, nw_p, T)
                build_rows(xT_local, lp, Tlocal)

    # ---- TileContext 2: attention + aggregation ----
    if "nomain" in ablate:
        w_pc = 0
    w_pc = min(w_pc, int(os.environ.get("K_WINCAP", 10**9)))
    with tile.TileContext(nc) as tc:
        with (
            tc.tile_pool(name="su2", bufs=1) as su,
            tc.tile_pool(name="ps_su2", bufs=1, space="PSUM") as psu,
            tc.tile_pool(name="sb_g", bufs=2) as sgp,
            tc.tile_pool(name="sb_w", bufs=2) as swp,
            tc.tile_pool(name="sb_sm", bufs=2) as ssp,
            tc.tile_pool(name="sb_out", bufs=2) as sop,
            tc.tile_pool(name="ps_mm", bufs=4, space="PSUM") as pmp,
        ):
            iota_t = su.tile([P, P], f16)
            nc.sync.dma_start(iota_t[:], iota_in[:])
            ones1 = su.tile([1, P], f32)
            nc.vector.memset(ones1[:], 1.0)
            bias_sb = su.tile([1, D], f32)
            nc.sync.dma_start(bias_sb[:], bias_in[:])
            bias5 = su.tile([P, D], f32)
            ps_bc = psu.tile([P, D], f32)
            nc.tensor.matmul(out=ps_bc[:], lhsT=ones1[:], rhs=bias_sb[:],
                             start=True, stop=True)
            nc.vector.tensor_scalar_mul(bias5[:], ps_bc[:], float(R))

            for w in range(w_pc):
                rows = slice(w * P, (w + 1) * P)
                lo_t = ssp.tile([P, NLO * 8], i16, name="lo_t")
                hi_t = ssp.tile([P, NHI * 8], i16, name="hi_t")
                ad_t = ssp.tile([P, NAD * 8], i16, name="ad_t")
                dlo_t = ssp.tile([P, NLO], f16, name="dlo_t")
                dhi_t = ssp.tile([P, NHI], f16, name="dhi_t")
                tself = ssp.tile([P, TU], f16, name="tself")
                nc.sync.dma_start(lo_t[:], lo16[rows, :])
                nc.sync.dma_start(hi_t[:], hi16[rows, :])
                nc.sync.dma_start(ad_t[:], ad16[rows, :])
                nc.sync.dma_start(dlo_t[:], dlo[rows, :])
                nc.sync.dma_start(dhi_t[:], dhi[rows, :])
                nc.sync.dma_start(tself[:], Tlocal[rows, 0:TU])

                G_lo = sgp.tile([P, NLO * TW], f16, name="G_lo")
                G_hi = sgp.tile([P, NHI * TW], f16, name="G_hi")
                G_a2 = sgp.tile([P, NAD * P], f16, name="G_a2")
                if "nogather" in ablate or "nolo" in ablate:
                    nc.vector.memset(G_lo[:], 0.25)
                else:
                    nc.gpsimd.dma_gather(
                        out_ap=G_lo[:].rearrange("p (j e) -> p j e", e=TW),
                        in_ap=T[0:low_cap, :],
                        idxs_ap=lo_t[:],
                        num_idxs=NLO * P, num_idxs_reg=NLO * P,
                        elem_size=TW, single_packet=False)
                if "nogather" in ablate or "nohi" in ablate:
                    nc.vector.memset(G_hi[:], 0.25)
                else:
                    nc.gpsimd.dma_gather(
                        out_ap=G_hi[:].rearrange("p (j e) -> p j e", e=TW),
                        in_ap=T[h0:t_rows, :],
                        idxs_ap=hi_t[:],
                        num_idxs=NHI * P, num_idxs_reg=NHI * P,
                        elem_size=TW, single_packet=False)
                if "nogather" in ablate or "noad" in ablate:
                    nc.vector.memset(G_a2[:], 0.25)
                else:
                    nc.gpsimd.dma_gather(
                        out_ap=G_a2[:].rearrange("p (j e) -> p j e", e=P),
                        in_ap=Tlocal[:, 0:P],
                        idxs_ap=ad_t[:],
                        num_idxs=NAD * P, num_idxs_reg=NAD * P,
                        elem_size=P, elem_step=TW, single_packet=False)

                # attention logits: asum = a_src(edge) + a_dst(dst)
                a2v = G_a2[:].rearrange("p (r b e) -> p r b e", r=R, e=P)
                asums = []
                for nm, G, nb, boff in (("lo", G_lo, B1, 0),
                                        ("hi", G_hi, B2, B1)):
                    asum = ssp.tile([P, R * nb * H], f16, name=f"as_{nm}")
                    nc.vector.tensor_tensor(
                        out=asum[:].rearrange("p (r b h) -> p r b h",
                                              r=R, h=H),
                        in0=G[:].rearrange("p (j e) -> p j e", e=TW)
                            [:, :, AS0:AS0 + H].rearrange(
                            "p (r b) h -> p r b h", r=R),
                        in1=a2v[:, :, boff:boff + nb, 0:H],
                        op=mybir.AluOpType.add)
                    lsc = ssp.tile([P, R * nb * H], f16, name=f"ls_{nm}")
                    nc.vector.tensor_scalar_mul(lsc[:], asum[:], NEG)
                    nc.vector.tensor_tensor(out=asum[:], in0=asum[:],
                                            in1=lsc[:],
                                            op=mybir.AluOpType.max)
                    nc.vector.tensor_scalar_min(asum[:], asum[:], CLAMP)
                    expl = ssp.tile([P, R * nb * H], f16, name=f"ex_{nm}")
                    nc.scalar.activation(expl[:], asum[:],
                                         mybir.ActivationFunctionType.Exp)
                    asums.append(expl)
                expl_lo, expl_hi = asums

                # one-hot Ind tiles for all blocks at once
                wt_lo = swp.tile([P, NLO * P], f16, name="wt_lo")
                wt_hi = swp.tile([P, NHI * P], f16, name="wt_hi")
                for wt, dt_, nb in ((wt_lo, dlo_t, NLO), (wt_hi, dhi_t, NHI)):
                    nc.vector.tensor_tensor(
                        out=wt[:].rearrange("p (j e) -> p j e", e=P),
                        in0=iota_t[:].unsqueeze(1).broadcast_to((P, nb, P)),
                        in1=dt_[:].unsqueeze(2).broadcast_to((P, nb, P)),
                        op=mybir.AluOpType.is_equal)

                # scale gathered rows in place by expl (per head)
                for G, expl, nb in ((G_lo, expl_lo, NLO),
                                    (G_hi, expl_hi, NHI)):
                    gsl = G[:].rearrange("p (j e) -> p j e", e=TW)[
                        :, :, FEAT0:FEAT0 + MMW].rearrange(
                        "p j (h x) -> p j h x", x=C + 1)
                    esl = expl[:].rearrange("p (j h) -> p j h", h=H
                                            ).unsqueeze(3).broadcast_to(
                        (P, nb, H, C + 1))
                    nc.vector.tensor_tensor(out=gsl, in0=gsl, in1=esl,
                                            op=mybir.AluOpType.mult)

                # per-relation matmuls: [num0|den0|num1|den1] in PSUM
                S = sop.tile([P, R * MMW], f32, name="S")
                for r in range(R):
                    ps = pmp.tile([P, MMW], f32, name="ps")
                    for b in range(BT):
                        if b < B1:
                            j = r * B1 + b
                            wt, G = wt_lo, G_lo
                        else:
                            j = r * B2 + (b - B1)
                            wt, G = wt_hi, G_hi
                        nc.tensor.matmul(
                            out=ps[:],
                            lhsT=wt[:, j * P:(j + 1) * P],
                            rhs=G[:, j * TW + FEAT0:j * TW + FEAT0 + MMW],
                            start=(b == 0), stop=(b == BT - 1))
                    nc.scalar.activation(S[:, r * MMW:(r + 1) * MMW], ps[:],
                                         mybir.ActivationFunctionType.Copy)

                # self-loop contribution (shared by all relations)
                asS = sop.tile([P, H], f32, name="asS")
                nc.vector.tensor_tensor(out=asS[:], in0=tself[:, AS0:AS0 + H],
                                        in1=tself[:, 0:H],
                                        op=mybir.AluOpType.add)
                lsS = sop.tile([P, H], f32, name="lsS")
                nc.vector.tensor_scalar_mul(lsS[:], asS[:], NEG)
                nc.vector.tensor_tensor(out=asS[:], in0=asS[:], in1=lsS[:],
                                        op=mybir.AluOpType.max)
                exS = sop.tile([P, H], f32, name="exS")
                nc.scalar.activation(exS[:], asS[:],
                                     mybir.ActivationFunctionType.Exp)
                tfF = sop.tile([P, D], f32, name="tfF")
                nc.vector.tensor_copy(
                    tfF[:].rearrange("p (h x) -> p h x", h=H),
                    tself[:, FEAT0:FEAT0 + MMW].rearrange(
                        "p (h x) -> p h x", x=C + 1)[:, :, 0:C])
                numS = sop.tile([P, D], f32, name="numS")
                nc.vector.tensor_tensor(
                    out=numS[:].rearrange("p (h x) -> p h x", h=H),
                    in0=tfF[:].rearrange("p (h x) -> p h x", h=H),
                    in1=exS[:].unsqueeze(2).broadcast_to((P, H, C)),
                    op=mybir.AluOpType.mult)

                # combine: (num + numS) / (den + exS) summed over rels + bias
                Sv = S[:].rearrange("p (r h x) -> p r h x", r=R, x=C + 1)
                dent = sop.tile([P, R * H], f32, name="dent")
                nc.vector.tensor_tensor(
                    out=dent[:].rearrange("p (r h) -> p r h", h=H),
                    in0=Sv[:, :, :, C],
                    in1=exS[:].unsqueeze(1).broadcast_to((P, R, H)),
                    op=mybir.AluOpType.add)
                recip = sop.tile([P, R * H], f32, name="recip")
                nc.vector.reciprocal(recip[:], dent[:])
                NT = sop.tile([P, R * D], f32, name="NT")
                nc.vector.tensor_tensor(
                    out=NT[:].rearrange("p (r h x) -> p r h x", r=R, x=C),
                    in0=Sv[:, :, :, 0:C],
                    in1=numS[:].rearrange("p (h x) -> p h x", h=H
                                          ).unsqueeze(1).broadcast_to(
                        (P, R, H, C)),
                    op=mybir.AluOpType.add)
                nc.vector.tensor_tensor(
                    out=NT[:].rearrange("p (r h x) -> p r h x", r=R, x=C),
                    in0=NT[:].rearrange("p (r h x) -> p r h x", r=R, x=C),
                    in1=recip[:].rearrange("p (r h) -> p r h", h=H
                                           ).unsqueeze(3).broadcast_to(
                        (P, R, H, C)),
                    op=mybir.AluOpType.mult)
                yacc = sop.tile([P, D], f32, name="yacc")
                nc.vector.tensor_reduce(
                    out=yacc[:],
                    in_=NT[:].rearrange("p (r x) -> p x r", r=R),
                    axis=mybir.AxisListType.X,
                    op=mybir.AluOpType.add)
                nc.vector.tensor_tensor(out=yacc[:], in0=yacc[:],
                                        in1=bias5[:],
                                        op=mybir.AluOpType.add)
                nc.sync.dma_start(y[rows, :], yacc[:])

    nc.finalize()
    return nc


def _wrap16(vals):
    """[n] int array -> 16-partition-wrapped [128, n//16] int16."""
    n = len(vals)
    assert n % 16 == 0
    a = np.asarray(vals, np.int16).reshape(n // 16, 16).T
    return np.tile(a, (8, 1))


def prep_inputs(inputs, ncores, low_cap=32768):
    x = np.asarray(inputs["x"], dtype=np.float32)
    N = x.shape[0]
    nw_real = -(-N // P)
    NW = -(-nw_real // ncores) * ncores
    w_pc = NW // ncores
    t_rows = NW * P
    nw_p = t_rows
    low_cap = min(low_cap, t_rows)
    h0 = t_rows - low_cap

    rels = ["parent", "child", "precede", "follow", "peer"]
    per_rel = []
    for rn in rels:
        ei = np.asarray(inputs[f"edge_index_{rn}"])
        src = ei[0].astype(np.int64)
        dst = ei[1].astype(np.int64)
        order = np.argsort(dst, kind="stable")
        src, dst = src[order], dst[order]
        cnt = np.bincount(dst // P, minlength=NW)
        starts = np.zeros(NW + 1, np.int64)
        np.cumsum(cnt, out=starts[1:])
        per_rel.append((src, dst, starts))

    must_lo_max = must_hi_max = tot_max = 0
    for src, dst, starts in per_rel:
        for w in range(NW):
            s, e = starts[w], starts[w + 1]
            sw = src[s:e]
            must_lo_max = max(must_lo_max, int((sw < h0).sum()))
            must_hi_max = max(must_hi_max, int((sw >= low_cap).sum()))
            tot_max = max(tot_max, e - s)
    B1 = max(1, -(-must_lo_max // P))
    B2 = max(1, -(-must_hi_max // P), -(-tot_max // P) - B1)
    while B1 * P < must_lo_max or (tot_max - B1 * P) > B2 * P:
        B1 += 1
    BT = B1 + B2

    lo_idx = np.zeros((NW, R, B1 * P), np.int64)
    hi_idx = np.zeros((NW, R, B2 * P), np.int64)
    ad_idx = np.zeros((NW, R, BT * P), np.int64)
    dlo_v = np.full((NW, R, B1 * P), float(P), np.float16)
    dhi_v = np.full((NW, R, B2 * P), float(P), np.float16)
    for r, (src, dst, starts) in enumerate(per_rel):
        for w in range(NW):
            s, e = starts[w], starts[w + 1]
            sw, dw = src[s:e], dst[s:e]
            is_lo = sw < h0
            is_hi = sw >= low_cap
            flex = ~is_lo & ~is_hi
            room = B1 * P - int(is_lo.sum())
            fi = np.flatnonzero(flex)
            lo_sel = np.concatenate([np.flatnonzero(is_lo), fi[:room]])
            hi_sel = np.concatenate([np.flatnonzero(is_hi), fi[room:]])
            assert len(lo_sel) <= B1 * P and len(hi_sel) <= B2 * P
            lo_idx[w, r, :len(lo_sel)] = sw[lo_sel]
            hi_idx[w, r, :len(hi_sel)] = sw[hi_sel] - h0
            ad_idx[w, r, :len(lo_sel)] = dw[lo_sel]
            ad_idx[w, r, B1 * P:B1 * P + len(hi_sel)] = dw[hi_sel]
            dlo_v[w, r, :len(lo_sel)] = (dw[lo_sel] - w * P)
            dhi_v[w, r, :len(hi_sel)] = (dw[hi_sel] - w * P)

    xTf = np.zeros((D, t_rows), np.float32)
    xTf[:, :N] = np.ascontiguousarray(x.T)

    shared = {
        "xT": xTf,
        "Wsrc": np.ascontiguousarray(np.asarray(inputs["W_src"], np.float32)),
        "Wdst": np.ascontiguousarray(np.asarray(inputs["W_dst"], np.float32)),
        "atts": np.asarray(inputs["att_src"], np.float32).reshape(1, D).copy(),
        "attd": np.asarray(inputs["att_dst"], np.float32).reshape(1, D).copy(),
        "bias_in": np.asarray(inputs["bias"], np.float32).reshape(1, D).copy(),
        "iota_in": np.tile(np.arange(P, dtype=np.float16), (P, 1)),
    }

    percore = []
    for c in range(ncores):
        cb = c * w_pc * P
        lo16 = np.zeros((w_pc * P, R * B1 * P // 16), np.int16)
        hi16 = np.zeros((w_pc * P, R * B2 * P // 16), np.int16)
        ad16 = np.zeros((w_pc * P, R * BT * P // 16), np.int16)
        dlo_a = np.zeros((w_pc * P, R * B1), np.float16)
        dhi_a = np.zeros((w_pc * P, R * B2), np.float16)
        ad_local = np.clip(ad_idx - cb, 0, w_pc * P - 1)
        for wl in range(w_pc):
            w = c * w_pc + wl
            rs = slice(wl * P, (wl + 1) * P)
            lo16[rs] = _wrap16(lo_idx[w].reshape(-1))
            hi16[rs] = _wrap16(hi_idx[w].reshape(-1))
            ad16[rs] = _wrap16(ad_local[w].reshape(-1))
            dlo_a[rs] = dlo_v[w].reshape(R * B1, P).T.reshape(P, R * B1)
            dhi_a[rs] = dhi_v[w].reshape(R * B2, P).T.reshape(P, R * B2)
        percore.append({
            "lo16": lo16, "hi16": hi16, "ad16": ad16,
            "dlo": dlo_a, "dhi": dhi_a,
            "xT_local": np.ascontiguousarray(xTf[:, cb:cb + w_pc * P]),
        })
    meta = dict(N=N, NW=NW, w_pc=w_pc, n_tiles=NW, t_rows=t_rows,
                B1=B1, B2=B2, low_cap=low_cap, h0=h0, nw_p=nw_p)
    return meta, shared, percore


def kernel(**inputs):
    global _LAST_RESULT
    from concourse.bass_utils import run_bass_kernel_spmd

    ncores = 8
    meta, shared, percore = prep_inputs(inputs, ncores)
    key = tuple(sorted(meta.items()))
    if key not in _CACHE:
        _CACHE[key] = build_program(
            meta["n_tiles"], meta["t_rows"], meta["w_pc"], meta["B1"],
            meta["B2"], meta["low_cap"], meta["h0"], meta["nw_p"], ncores)
    nc = _CACHE[key]
    in_maps = [dict(shared, **percore[c]) for c in range(ncores)]
    res = run_bass_kernel_spmd(nc, in_maps, core_ids=list(range(ncores)),
                               **_RUN_KWARGS)
    _LAST_RESULT = res
    out = np.concatenate([res.results[c]["y"] for c in range(ncores)], axis=0)
    return np.ascontiguousarray(out[:meta["N"]])


# revision 11
# speedup vs baseline: 1.9893x; 1.5802x over previous
"""Trainium2 Bass kernel for 5-relation GAT (nn_GAT_76716705841462), v2.

Destination-sharded, collective-free, fp16 node table.

Node table T row (fp16, 384 elems = 768B stride, mult of 256B):
  [ad0 ad1 | h0 (2:130) | 1 (130) | h1 (131:259) | 1 (259) | as0 as1 | pad]
One row layout serves three consumers:
  * main gather (768B/edge): features+ones+a_src (+unused ad) by global src,
    split lo/hi so int16 indices fit;
  * a_dst gather (256B/edge): first 256B of the row via elem_step=384 from
    the per-core local slice Tlocal, window-local int16 indices;
  * self-loops: plain DMA of the window's own 128 rows (Tself).
Self-loops are excluded from the edge lists: their contribution (same for
all 5 relations) is computed once per window from Tself and added to every
relation's numerator/denominator.

Per (window, rel): one-hot Ind[e, dst] built for ALL blocks in one DVE
is_equal with broadcast APs; gathered rows scaled in-place by
exp(min(leaky_relu(a_src+a_dst), CLAMP)) (per-head); one 258-wide matmul
per 128-edge block accumulates [num_h0|den_h0|num_h1|den_h1] in PSUM
(denominator rides the embedded "1" columns). Final: den += explS,
reciprocal, num += numS, alpha-scale, reduce over rels, +5*bias.
"""

import numpy as np

import concourse.bacc as bacc
import concourse.bass as bass
import concourse.mybir as mybir
import concourse.tile as tile

P = 128
H = 2
C = 128
D = 256
R = 5
TW = 384          # T row width (fp16): 768B
TU = 262          # used cols
FEAT0 = 2         # h0 at 2:130, one at 130, h1 at 131:259, one at 259
AS0 = 260         # a_src at 260:262
MMW = 2 * (C + 1)  # 258-wide matmul rhs [h0|1|h1|1]
CLAMP = 10.5
NEG = 0.2

f32 = mybir.dt.float32
f16 = mybir.dt.float16
f8 = mybir.dt.float8e4
i16 = mybir.dt.int16

_CACHE = {}
_RUN_KWARGS = {}
_LAST_RESULT = None


def build_program(n_tiles, t_rows, w_pc, B1, B2, low_cap, h0, nw_p,
                  num_devices):
    import os
    ablate = set(os.environ.get("K_ABLATE", "").split(","))
    BT = B1 + B2
    NLO = R * B1          # lo blocks per window
    NHI = R * B2
    NAD = R * BT
    lp = w_pc * P         # local rows per core
    nc = bacc.Bacc("TRN2", target_bir_lowering=False, debug=False,
                   num_devices=num_devices)

    xT = nc.dram_tensor("xT", [D, nw_p], f32, kind="ExternalInput")
    xT_local = nc.dram_tensor("xT_local", [D, lp], f32, kind="ExternalInput")
    Wsrc = nc.dram_tensor("Wsrc", [D, D], f32, kind="ExternalInput")
    Wdst = nc.dram_tensor("Wdst", [D, D], f32, kind="ExternalInput")
    atts = nc.dram_tensor("atts", [1, D], f32, kind="ExternalInput")
    attd = nc.dram_tensor("attd", [1, D], f32, kind="ExternalInput")
    bias_in = nc.dram_tensor("bias_in", [1, D], f32, kind="ExternalInput")
    lo16 = nc.dram_tensor("lo16", [lp, NLO * P // 16], i16,
                          kind="ExternalInput")
    hi16 = nc.dram_tensor("hi16", [lp, NHI * P // 16], i16,
                          kind="ExternalInput")
    wt8 = nc.dram_tensor("wt8", [lp, (NLO + NHI) * P], f8,
                         kind="ExternalInput")
    it8 = nc.dram_tensor("it8", [lp, NAD * P], f8,
                         kind="ExternalInput")
    y = nc.dram_tensor("y", [lp, D], f32, kind="ExternalOutput")

    T = nc.dram_tensor("T", [t_rows, TW], f16)
    Tlocal = nc.dram_tensor("Tlocal", [lp, TW], f16)

    # ---- TileContext 1: weights setup + node tables ----
    with tile.TileContext(nc) as tc:
        with (
            tc.tile_pool(name="setup", bufs=1) as su,
            tc.tile_pool(name="ps_su", bufs=1, space="PSUM") as psu,
        ):
            ws_h = [su.tile([P, D], f32, name=f"ws_h{k}") for k in range(2)]
            wd_h = [su.tile([P, D], f32, name=f"wd_h{k}") for k in range(2)]
            for k in range(2):
                nc.sync.dma_start(ws_h[k][:], Wsrc[k * P:(k + 1) * P, :])
                nc.sync.dma_start(wd_h[k][:], Wdst[k * P:(k + 1) * P, :])
            ones1 = su.tile([1, P], f32)
            nc.vector.memset(ones1[:], 1.0)
            atts_sb = su.tile([1, D], f32)
            attd_sb = su.tile([1, D], f32)
            nc.sync.dma_start(atts_sb[:], atts[:])
            nc.sync.dma_start(attd_sb[:], attd[:])
            atts_bc = su.tile([P, D], f32)
            attd_bc = su.tile([P, D], f32)
            for row_sb, bc in ((atts_sb, atts_bc), (attd_sb, attd_bc)):
                ps_bc = psu.tile([P, D], f32, name="ps_bc", tag="ps_bc")
                nc.tensor.matmul(out=ps_bc[:], lhsT=ones1[:], rhs=row_sb[:],
                                 start=True, stop=True)
                nc.vector.tensor_copy(bc[:], ps_bc[:])

            # rhs16_k[k]: [P, TU] fp16 matmul rhs producing one T row tile
            rhs16_k = []
            for k in range(2):
                r32 = su.tile([P, TU], f32, name=f"r32_{k}", tag="r32",
                              bufs=2)
                nc.vector.memset(r32[:], 0.0)
                nc.vector.tensor_copy(r32[:, FEAT0:FEAT0 + C],
                                      ws_h[k][:, 0:C])
                nc.vector.tensor_copy(r32[:, FEAT0 + C + 1:FEAT0 + 2 * C + 1],
                                      ws_h[k][:, C:D])
                for h in range(H):
                    for src_w, src_bc, col in (
                        (ws_h[k], atts_bc, AS0 + h),
                        (wd_h[k], attd_bc, h),
                    ):
                        scratch = su.tile([P, C], f32, name="vscr",
                                          tag="vscr", bufs=2)
                        nc.vector.tensor_tensor(
                            out=scratch[:],
                            in0=src_w[:, h * C:(h + 1) * C],
                            in1=src_bc[:, h * C:(h + 1) * C],
                            op=mybir.AluOpType.mult)
                        nc.vector.tensor_reduce(
                            out=r32[:, col:col + 1], in_=scratch[:],
                            axis=mybir.AxisListType.X,
                            op=mybir.AluOpType.add)
                r16 = su.tile([P, TU], f16, name=f"rhs16_{k}")
                nc.vector.tensor_copy(r16[:], r32[:])
                rhs16_k.append(r16)

            # node table build: global T then local Tlocal
            with (
                tc.tile_pool(name="sb_tbl", bufs=3) as stp,
                tc.tile_pool(name="ps_tbl", bufs=4, space="PSUM") as ptp,
            ):
                def build_rows(src_dram, ncols, out_dram):
                    ntile = ncols // P
                    CH = 16       # tiles per strip
                    for c0 in range(0, ntile, CH):
                        cn = min(CH, ntile - c0)
                        w0, w1 = c0 * P, (c0 + cn) * P
                        s32a = stp.tile([P, CH * P], f32, name="s32a")
                        s32b = stp.tile([P, CH * P], f32, name="s32b")
                        nc.sync.dma_start(s32a[:, :cn * P],
                                          src_dram[0:P, w0:w1])
                        nc.sync.dma_start(s32b[:, :cn * P],
                                          src_dram[P:D, w0:w1])
                        s16a = stp.tile([P, CH * P], f16, name="s16a")
                        s16b = stp.tile([P, CH * P], f16, name="s16b")
                        nc.vector.tensor_copy(s16a[:, :cn * P],
                                              s32a[:, :cn * P])
                        nc.vector.tensor_copy(s16b[:, :cn * P],
                                              s32b[:, :cn * P])
                        for g0 in range(0, cn, 4):
                            gn = min(4, cn - g0)
                            stg4 = stp.tile([P, 4 * TW], f16, name="stg4")
                            nc.vector.memset(
                                stg4[:].rearrange("p (j e) -> p j e", e=TW)
                                [:, 0:gn, TU:TW], 0.0)
                            for j in range(gn):
                                t = g0 + j
                                ps_t = ptp.tile([P, TU], f32, name="ps_t")
                                nc.tensor.matmul(
                                    out=ps_t[:],
                                    lhsT=s16a[:, t * P:(t + 1) * P],
                                    rhs=rhs16_k[0][:],
                                    start=True, stop=False)
                                nc.tensor.matmul(
                                    out=ps_t[:],
                                    lhsT=s16b[:, t * P:(t + 1) * P],
                                    rhs=rhs16_k[1][:],
                                    start=False, stop=True)
                                nc.scalar.activation(
                                    stg4[:, j * TW:j * TW + TU], ps_t[:],
                                    mybir.ActivationFunctionType.Copy)
                            ones_ap = stg4[:].rearrange(
                                "p (j e) -> p j e", e=TW)[
                                :, 0:gn, FEAT0 + C:FEAT0 + 2 * C + 2:C + 1]
                            nc.vector.memset(ones_ap, 1.0)
                            r0 = (c0 + g0) * P
                            out_ap = out_dram[
                                r0:r0 + gn * P, :].rearrange(
                                "(j p) e -> p j e", p=P)
                            in_ap = stg4[:, :gn * TW].rearrange(
                                "p (j e) -> p j e", e=TW)
                            nc.sync.dma_start(out_ap, in_ap)

                build_rows(xT, nw_p, T)
                build_rows(xT_local, lp, Tlocal)

    # ---- TileContext 2: attention + aggregation ----
    if "nomain" in ablate:
        w_pc = 0
    w_pc = min(w_pc, int(os.environ.get("K_WINCAP", 10**9)))
    with tile.TileContext(nc) as tc:
        with (
            tc.tile_pool(name="su2", bufs=1) as su,
            tc.tile_pool(name="ps_su2", bufs=1, space="PSUM") as psu,
            tc.tile_pool(name="sb_g", bufs=2) as sgp,
            tc.tile_pool(name="sb_w", bufs=2) as swp,
            tc.tile_pool(name="sb_sm", bufs=2) as ssp,
            tc.tile_pool(name="sb_out", bufs=2) as sop,
            tc.tile_pool(name="ps_mm", bufs=3, space="PSUM") as pmp,
            tc.tile_pool(name="ps_ad", bufs=2, space="PSUM") as pap,
        ):
            ones1 = su.tile([1, P], f32)
            nc.vector.memset(ones1[:], 1.0)
            bias_sb = su.tile([1, D], f32)
            nc.sync.dma_start(bias_sb[:], bias_in[:])
            bias5 = su.tile([P, D], f32)
            ps_bc = psu.tile([P, D], f32)
            nc.tensor.matmul(out=ps_bc[:], lhsT=ones1[:], rhs=bias_sb[:],
                             start=True, stop=True)
            nc.vector.tensor_scalar_mul(bias5[:], ps_bc[:], float(R))
            NB = NLO + NHI

            for w in range(w_pc):
                rows = slice(w * P, (w + 1) * P)
                lo_t = ssp.tile([P, NLO * 8], i16, name="lo_t")
                hi_t = ssp.tile([P, NHI * 8], i16, name="hi_t")
                wt8_t = ssp.tile([P, NB * P], f8, name="wt8_t")
                it8_t = ssp.tile([P, NAD * P], f8, name="it8_t")
                tself = ssp.tile([P, TU], f16, name="tself")
                nc.sync.dma_start(lo_t[:], lo16[rows, :])
                nc.sync.dma_start(hi_t[:], hi16[rows, :])
                nc.sync.dma_start(wt8_t[:], wt8[rows, :])
                nc.sync.dma_start(it8_t[:], it8[rows, :])
                nc.sync.dma_start(tself[:], Tlocal[rows, 0:TU])
                wt16 = swp.tile([P, NB * P], f16, name="wt16")
                it16 = swp.tile([P, NAD * P], f16, name="it16")
                nc.vector.tensor_copy(wt16[:], wt8_t[:])
                nc.vector.tensor_copy(it16[:], it8_t[:])

                G_lo = sgp.tile([P, NLO * TW], f16, name="G_lo")
                G_hi = sgp.tile([P, NHI * TW], f16, name="G_hi")
                if "nogather" in ablate or "nolo" in ablate:
                    nc.vector.memset(G_lo[:], 0.25)
                else:
                    nc.gpsimd.dma_gather(
                        out_ap=G_lo[:].rearrange("p (j e) -> p j e", e=TW),
                        in_ap=T[0:low_cap, :],
                        idxs_ap=lo_t[:],
                        num_idxs=NLO * P, num_idxs_reg=NLO * P,
                        elem_size=TW, single_packet=False)
                if "nogather" in ablate or "nohi" in ablate:
                    nc.vector.memset(G_hi[:], 0.25)
                else:
                    nc.gpsimd.dma_gather(
                        out_ap=G_hi[:].rearrange("p (j e) -> p j e", e=TW),
                        in_ap=T[h0:t_rows, :],
                        idxs_ap=hi_t[:],
                        num_idxs=NHI * P, num_idxs_reg=NHI * P,
                        elem_size=TW, single_packet=False)

                # a_dst per edge slot via tiny expand matmuls (lhsT = one-hot
                # transposed, rhs = this window's a_dst pairs)
                adC = ssp.tile([P, NAD * H], f16, name="adC")
                for r in range(R):
                    ps_ad = pap.tile([P, BT * H], f32, name="ps_ad")
                    for b in range(BT):
                        nc.tensor.matmul(
                            out=ps_ad[:, b * H:(b + 1) * H],
                            lhsT=it16[:, (r * BT + b) * P:(r * BT + b + 1) * P],
                            rhs=tself[:, 0:H],
                            start=True, stop=True)
                    nc.scalar.activation(
                        adC[:, r * B1 * H:(r + 1) * B1 * H],
                        ps_ad[:, 0:B1 * H],
                        mybir.ActivationFunctionType.Copy)
                    nc.scalar.activation(
                        adC[:, NLO * H + r * B2 * H:NLO * H + (r + 1) * B2 * H],
                        ps_ad[:, B1 * H:BT * H],
                        mybir.ActivationFunctionType.Copy)

                # a_src per edge slot: strided SBUF->SBUF extract
                asLH = ssp.tile([P, NB * H], f16, name="asLH")
                nc.sync.dma_start(
                    asLH[:, 0:NLO * H].rearrange("p (j h) -> p j h", h=H),
                    G_lo[:].rearrange("p (j e) -> p j e", e=TW)
                    [:, :, AS0:AS0 + H])
                nc.sync.dma_start(
                    asLH[:, NLO * H:].rearrange("p (j h) -> p j h", h=H),
                    G_hi[:].rearrange("p (j e) -> p j e", e=TW)
                    [:, :, AS0:AS0 + H])

                # asum -> leaky relu -> clamp -> exp   (all contiguous)
                asm = ssp.tile([P, (NB + 1) * H], f16, name="asm")
                nc.vector.tensor_tensor(out=asm[:, 0:NB * H], in0=asLH[:],
                                        in1=adC[:], op=mybir.AluOpType.add)
                nc.vector.tensor_tensor(out=asm[:, NB * H:],
                                        in0=tself[:, AS0:AS0 + H],
                                        in1=tself[:, 0:H],
                                        op=mybir.AluOpType.add)
                lsc = ssp.tile([P, (NB + 1) * H], f16, name="lsc")
                nc.vector.tensor_scalar_mul(lsc[:], asm[:], NEG)
                nc.vector.tensor_tensor(out=asm[:], in0=asm[:], in1=lsc[:],
                                        op=mybir.AluOpType.max)
                nc.vector.tensor_scalar_min(asm[:], asm[:], CLAMP)
                expl = ssp.tile([P, NB * H], f16, name="expl")
                nc.scalar.activation(expl[:], asm[:, 0:NB * H],
                                     mybir.ActivationFunctionType.Exp)
                exS = sop.tile([P, H], f32, name="exS")
                nc.scalar.activation(exS[:], asm[:, NB * H:],
                                     mybir.ActivationFunctionType.Exp)

                # scale gathered rows in place by expl (per head)
                for G, off, nb in ((G_lo, 0, NLO), (G_hi, NLO, NHI)):
                    gsl = G[:].rearrange("p (j e) -> p j e", e=TW)[
                        :, :, FEAT0:FEAT0 + MMW].rearrange(
                        "p j (h x) -> p j h x", x=C + 1)
                    esl = expl[:, off * H:(off + nb) * H].rearrange(
                        "p (j h) -> p j h", h=H).unsqueeze(3).broadcast_to(
                        (P, nb, H, C + 1))
                    nc.vector.tensor_tensor(out=gsl, in0=gsl, in1=esl,
                                            op=mybir.AluOpType.mult)

                # per-relation matmuls: [num0|den0|num1|den1] in PSUM
                S = sop.tile([P, R * MMW], f32, name="S")
                for r in range(R):
                    ps = pmp.tile([P, MMW], f32, name="ps")
                    for b in range(BT):
                        if b < B1:
                            j = r * B1 + b
                            G = G_lo
                        else:
                            j = NLO + r * B2 + (b - B1)
                            G = G_hi
                        gj = j if b < B1 else j - NLO
                        nc.tensor.matmul(
                            out=ps[:],
                            lhsT=wt16[:, j * P:(j + 1) * P],
                            rhs=G[:, gj * TW + FEAT0:gj * TW + FEAT0 + MMW],
                            start=(b == 0), stop=(b == BT - 1))
                    nc.scalar.activation(S[:, r * MMW:(r + 1) * MMW], ps[:],
                                         mybir.ActivationFunctionType.Copy)

                # combine: alpha-scale on ACT, reduce over rels (+self), bias
                dent = sop.tile([P, R * H], f32, name="dent")
                nc.sync.dma_start(
                    dent[:].rearrange("p (r h) -> p r h", h=H),
                    S[:].rearrange("p (r h x) -> p r h x", r=R, x=C + 1)
                    [:, :, :, C])
                exS5 = sop.tile([P, R * H], f32, name="exS5")
                nc.vector.tensor_copy(
                    exS5[:].rearrange("p (r h) -> p r h", h=H),
                    exS[:].unsqueeze(1).broadcast_to((P, R, H)))
                nc.vector.tensor_tensor(out=dent[:], in0=dent[:],
                                        in1=exS5[:], op=mybir.AluOpType.add)
                recip = sop.tile([P, R * H], f32, name="recip")
                nc.vector.reciprocal(recip[:], dent[:])
                rsum = sop.tile([P, H], f32, name="rsum")
                nc.vector.tensor_reduce(
                    out=rsum[:],
                    in_=recip[:].rearrange("p (r h) -> p h r", h=H),
                    axis=mybir.AxisListType.X, op=mybir.AluOpType.add)
                sc = sop.tile([P, H], f32, name="sc")
                nc.vector.tensor_tensor(out=sc[:], in0=exS[:], in1=rsum[:],
                                        op=mybir.AluOpType.mult)
                AM = sop.tile([P, (R + 1) * D], f32, name="AM")
                for r in range(R):
                    for h in range(H):
                        nc.scalar.activation(
                            AM[:, r * D + h * C:r * D + (h + 1) * C],
                            S[:, r * MMW + h * (C + 1):
                              r * MMW + h * (C + 1) + C],
                            mybir.ActivationFunctionType.Copy,
                            scale=recip[:, r * H + h:r * H + h + 1])
                for h in range(H):
                    nc.scalar.activation(
                        AM[:, R * D + h * C:R * D + (h + 1) * C],
                        tself[:, FEAT0 + h * (C + 1):
                              FEAT0 + h * (C + 1) + C],
                        mybir.ActivationFunctionType.Copy,
                        scale=sc[:, h:h + 1])
                yacc = sop.tile([P, D], f32, name="yacc")
                nc.vector.tensor_reduce(
                    out=yacc[:],
                    in_=AM[:].rearrange("p (r x) -> p x r", r=R + 1),
                    axis=mybir.AxisListType.X, op=mybir.AluOpType.add)
                nc.vector.tensor_tensor(out=yacc[:], in0=yacc[:],
                                        in1=bias5[:],
                                        op=mybir.AluOpType.add)
                nc.sync.dma_start(y[rows, :], yacc[:])

    nc.finalize()
    return nc


def _wrap16(vals):
    """[n] int array -> 16-partition-wrapped [128, n//16] int16."""
    n = len(vals)
    assert n % 16 == 0
    a = np.asarray(vals, np.int16).reshape(n // 16, 16).T
    return np.tile(a, (8, 1))


def prep_inputs(inputs, ncores, low_cap=32768):
    x = np.asarray(inputs["x"], dtype=np.float32)
    N = x.shape[0]
    nw_real = -(-N // P)
    NW = -(-nw_real // ncores) * ncores
    w_pc = NW // ncores
    t_rows = NW * P
    nw_p = t_rows
    low_cap = min(low_cap, t_rows)
    h0 = t_rows - low_cap

    rels = ["parent", "child", "precede", "follow", "peer"]
    per_rel = []
    for rn in rels:
        ei = np.asarray(inputs[f"edge_index_{rn}"])
        src = ei[0].astype(np.int64)
        dst = ei[1].astype(np.int64)
        order = np.argsort(dst, kind="stable")
        src, dst = src[order], dst[order]
        cnt = np.bincount(dst // P, minlength=NW)
        starts = np.zeros(NW + 1, np.int64)
        np.cumsum(cnt, out=starts[1:])
        per_rel.append((src, dst, starts))

    must_lo_max = must_hi_max = tot_max = 0
    for src, dst, starts in per_rel:
        for w in range(NW):
            s, e = starts[w], starts[w + 1]
            sw = src[s:e]
            must_lo_max = max(must_lo_max, int((sw < h0).sum()))
            must_hi_max = max(must_hi_max, int((sw >= low_cap).sum()))
            tot_max = max(tot_max, e - s)
    B1 = max(1, -(-must_lo_max // P))
    B2 = max(1, -(-must_hi_max // P), -(-tot_max // P) - B1)
    while B1 * P < must_lo_max or (tot_max - B1 * P) > B2 * P:
        B1 += 1
    BT = B1 + B2

    lo_idx = np.zeros((NW, R, B1 * P), np.int64)
    hi_idx = np.zeros((NW, R, B2 * P), np.int64)
    ad_idx = np.zeros((NW, R, BT * P), np.int64)
    dlo_v = np.full((NW, R, B1 * P), float(P), np.float16)
    dhi_v = np.full((NW, R, B2 * P), float(P), np.float16)
    for r, (src, dst, starts) in enumerate(per_rel):
        for w in range(NW):
            s, e = starts[w], starts[w + 1]
            sw, dw = src[s:e], dst[s:e]
            is_lo = sw < h0
            is_hi = sw >= low_cap
            flex = ~is_lo & ~is_hi
            room = B1 * P - int(is_lo.sum())
            fi = np.flatnonzero(flex)
            lo_sel = np.concatenate([np.flatnonzero(is_lo), fi[:room]])
            hi_sel = np.concatenate([np.flatnonzero(is_hi), fi[room:]])
            assert len(lo_sel) <= B1 * P and len(hi_sel) <= B2 * P
            lo_idx[w, r, :len(lo_sel)] = sw[lo_sel]
            hi_idx[w, r, :len(hi_sel)] = sw[hi_sel] - h0
            ad_idx[w, r, :len(lo_sel)] = dw[lo_sel]
            ad_idx[w, r, B1 * P:B1 * P + len(hi_sel)] = dw[hi_sel]
            dlo_v[w, r, :len(lo_sel)] = (dw[lo_sel] - w * P)
            dhi_v[w, r, :len(hi_sel)] = (dw[hi_sel] - w * P)

    xTf = np.zeros((D, t_rows), np.float32)
    xTf[:, :N] = np.ascontiguousarray(x.T)

    shared = {
        "xT": xTf,
        "Wsrc": np.ascontiguousarray(np.asarray(inputs["W_src"], np.float32)),
        "Wdst": np.ascontiguousarray(np.asarray(inputs["W_dst"], np.float32)),
        "atts": np.asarray(inputs["att_src"], np.float32).reshape(1, D).copy(),
        "attd": np.asarray(inputs["att_dst"], np.float32).reshape(1, D).copy(),
        "bias_in": np.asarray(inputs["bias"], np.float32).reshape(1, D).copy(),
    }

    import ml_dtypes
    f8np = ml_dtypes.float8_e4m3fn
    NLO, NHI, NAD = R * B1, R * B2, R * BT
    nids = np.arange(P)
    percore = []
    for c in range(ncores):
        cb = c * w_pc * P
        lo16 = np.zeros((w_pc * P, NLO * P // 16), np.int16)
        hi16 = np.zeros((w_pc * P, NHI * P // 16), np.int16)
        wt8 = np.zeros((w_pc * P, (NLO + NHI) * P), f8np)
        it8 = np.zeros((w_pc * P, NAD * P), f8np)
        for wl in range(w_pc):
            w = c * w_pc + wl
            rs = slice(wl * P, (wl + 1) * P)
            lo16[rs] = _wrap16(lo_idx[w].reshape(-1))
            hi16[rs] = _wrap16(hi_idx[w].reshape(-1))
            # wt: [e-part, (lo blocks | hi blocks) x dst] one-hot
            vlo = dlo_v[w].reshape(NLO, P)          # [j, e]
            vhi = dhi_v[w].reshape(NHI, P)
            oh_lo = (vlo[:, :, None] == nids)        # [j, e, n]
            oh_hi = (vhi[:, :, None] == nids)
            wt8[rs, :NLO * P] = oh_lo.transpose(1, 0, 2).reshape(P, -1)
            wt8[rs, NLO * P:] = oh_hi.transpose(1, 0, 2).reshape(P, -1)
            # it: [n-part, (r: lo blocks then hi blocks) x e] one-hot
            vv = np.concatenate(
                [np.concatenate([vlo[r * B1:(r + 1) * B1],
                                 vhi[r * B2:(r + 1) * B2]])
                 for r in range(R)])                 # [NAD, P] = [j, e]
            oh = (vv[:, :, None] == nids)            # [j, e, n]
            it8[rs] = oh.transpose(2, 0, 1).reshape(P, -1)
        percore.append({
            "lo16": lo16, "hi16": hi16, "wt8": wt8, "it8": it8,
            "xT_local": np.ascontiguousarray(xTf[:, cb:cb + w_pc * P]),
        })
    meta = dict(N=N, NW=NW, w_pc=w_pc, n_tiles=NW, t_rows=t_rows,
                B1=B1, B2=B2, low_cap=low_cap, h0=h0, nw_p=nw_p)
    return meta, shared, percore


def kernel(**inputs):
    global _LAST_RESULT
    from concourse.bass_utils import run_bass_kernel_spmd

    ncores = 8
    meta, shared, percore = prep_inputs(inputs, ncores)
    key = tuple(sorted(meta.items()))
    if key not in _CACHE:
        _CACHE[key] = build_program(
            meta["n_tiles"], meta["t_rows"], meta["w_pc"], meta["B1"],
            meta["B2"], meta["low_cap"], meta["h0"], meta["nw_p"], ncores)
    nc = _CACHE[key]
    in_maps = [dict(shared, **percore[c]) for c in range(ncores)]
    res = run_bass_kernel_spmd(nc, in_maps, core_ids=list(range(ncores)),
                               **_RUN_KWARGS)
    _LAST_RESULT = res
    out = np.concatenate([res.results[c]["y"] for c in range(ncores)], axis=0)
    return np.ascontiguousarray(out[:meta["N"]])
